# revision 6
# baseline (speedup 1.0000x reference)
"""Trainium2 Bass kernel for a 3-layer stacked LSTM step + output projection.

Problem: B=256, I=512, H=1024, V=512 (see reference):
    h0n,c0n = LSTMCell(x,  h0,c0; W0,U0,b0)
    h1n,c1n = LSTMCell(h0n,h1,c1; W1,U1,b1)
    h2n,c2n = LSTMCell(h1n,h2,c2; W2,U2,b2)
    logits  = h2n @ Wp + bp
    returns (logits, h0n, c0n, h1n, c1n, h2n, c2n)

Distribution: tensor-parallel over the gate dimension on 8 NeuronCores.
Core k owns h-columns [k*128,(k+1)*128) of every gate (weight slices of
width 4*128=512, gate-order [g|i|f|o] so tanh/sigmoid each cover one
contiguous strip). h_new is produced transposed, all-gathered in fp16
across cores between layers (2 AllGathers; a tiny warmup AllGather is
issued first to pay the collective-stream bootstrap early). The final
projection is K-parallel: each core multiplies its local h2n^T slice by
its 128 rows of Wp and the host sums the 8 partial logits.

Matmuls run in fp16 (weights/activations cast on host; ~1e-3 rel err vs
the fp32 reference). All elementwise math, cell states and outputs stay
fp32.
"""
import os
import sys
import numpy as np

sys.path.insert(0, '/opt/trn_rl_repo')

from contextlib import ExitStack

B, I, H, V = 256, 512, 1024, 512
N_CORES = 8
P = 128
GL = H // N_CORES          # gate cols owned per core = 128
LW = 4 * GL                # local gate width = 512
MB = B // P                # batch tiles = 2
KI = I // P                # x K-chunks = 4
KH = H // P                # h K-chunks = 8

_compiled = None


def _build():
    import concourse.bass as bass
    import concourse.tile as tile
    from concourse import bacc, mybir

    f16 = mybir.dt.float16
    f32 = mybir.dt.float32

    nc = bacc.Bacc("TRN2", target_bir_lowering=False, debug=False,
                   num_devices=N_CORES)

    # ---- DRAM I/O (per-core tensors, SPMD) ----
    # inputs, all host-prepped into partition-major layouts
    xT_d = nc.dram_tensor("xT", [P, KI * B], f16, kind="ExternalInput").ap()
    hT_d = [nc.dram_tensor(f"h{l}T", [P, KH * B], f16, kind="ExternalInput").ap()
            for l in range(3)]
    c_d = [nc.dram_tensor(f"c{l}", [P, MB * GL], f32, kind="ExternalInput").ap()
           for l in range(3)]
    W_d = [nc.dram_tensor(f"W{l}", [P, (KI if l == 0 else KH) * LW], f16,
                          kind="ExternalInput").ap() for l in range(3)]
    U_d = [nc.dram_tensor(f"U{l}", [P, KH * LW], f16, kind="ExternalInput").ap()
           for l in range(3)]
    b_d = [nc.dram_tensor(f"b{l}", [1, LW], f16, kind="ExternalInput").ap()
           for l in range(3)]
    Wp_d = nc.dram_tensor("Wp", [P, V], f16, kind="ExternalInput").ap()

    # outputs
    hn_d = [nc.dram_tensor(f"h{l}n", [P, MB * GL], f32, kind="ExternalOutput").ap()
            for l in range(3)]
    cn_d = [nc.dram_tensor(f"c{l}n", [P, MB * GL], f32, kind="ExternalOutput").ap()
            for l in range(3)]
    logits_d = nc.dram_tensor("logits", [B, V], f32, kind="ExternalOutput").ap()

    # collective bounce buffers
    warm_in = nc.dram_tensor("warm_in", [1, 16], f32)
    warm_out = nc.dram_tensor("warm_out", [N_CORES, 16], f32, addr_space="Shared")
    ag_in = [nc.dram_tensor(f"ag{l}_in", [P, B], f16) for l in range(2)]
    ag_out = [nc.dram_tensor(f"ag{l}_out", [N_CORES * P, B], f16, addr_space="Shared")
              for l in range(2)]

    RG = [list(range(N_CORES))]

    with tile.TileContext(nc) as tc:
        with ExitStack() as ctx:
            sbuf = ctx.enter_context(tc.tile_pool(name="sbuf", bufs=1))
            psum = ctx.enter_context(tc.tile_pool(name="psum", bufs=4, space="PSUM"))
            psum_tp = ctx.enter_context(tc.tile_pool(name="psum_tp", bufs=2, space="PSUM"))

            # -- warmup collective: pay the cc-stream bootstrap right away
            wt = sbuf.tile([1, 16], f32, name="wt")
            nc.gpsimd.memset(wt[:], 0.0)
            nc.sync.dma_start(warm_in.ap()[:, :], wt[:])
            nc.gpsimd.collective_compute(
                "AllGather", mybir.AluOpType.bypass, replica_groups=RG,
                ins=[warm_in.ap().opt()], outs=[warm_out.ap().opt()])

            # -- constants
            ones = sbuf.tile([1, P], f16, name="ones")
            nc.gpsimd.memset(ones[:], 1.0)
            from concourse.masks import make_identity
            ident = sbuf.tile([P, P], f16, name="ident")
            make_identity(nc, ident[:])

            # -- load cell0 inputs
            xT = sbuf.tile([P, KI * B], f16, name="xT")
            nc.sync.dma_start(xT[:], xT_d[:, :])
            hT = []
            for l in range(3):
                t = sbuf.tile([P, KH * B], f16, name=f"hT{l}")
                nc.sync.dma_start(t[:], hT_d[l][:, :])
                hT.append(t)
            W = []
            for l in range(3):
                kw = KI if l == 0 else KH
                t = sbuf.tile([P, kw * LW], f16, name=f"W{l}")
                nc.sync.dma_start(t[:], W_d[l][:, :])
                W.append(t)
            U = []
            for l in range(3):
                t = sbuf.tile([P, KH * LW], f16, name=f"U{l}")
                nc.sync.dma_start(t[:], U_d[l][:, :])
                U.append(t)
            bb = []
            for l in range(3):
                t = sbuf.tile([1, LW], f16, name=f"b{l}")
                nc.sync.dma_start(t[:], b_d[l][:, :])
                bb.append(t)
            cc = []
            for l in range(3):
                t = sbuf.tile([P, MB * GL], f32, name=f"c{l}")
                nc.sync.dma_start(t[:], c_d[l][:, :])
                cc.append(t)
            Wp = sbuf.tile([P, V], f16, name="Wp")
            nc.sync.dma_start(Wp[:], Wp_d[:, :])

            agout_sb = []  # gathered h^T per layer boundary, fp16 [128, KH*B]

            def cell(l, z_ps):
                """Activations, state update, output DMA, transpose, AG."""
                hn16 = []
                for m in range(MB):
                    zp = z_ps[m]
                    g_sb = sbuf.tile([P, GL], f32, name=f"g_{l}_{m}")
                    nc.scalar.activation(g_sb[:], zp[:, 0:GL],
                                         mybir.ActivationFunctionType.Tanh)
                    s_sb = sbuf.tile([P, 3 * GL], f32, name=f"s_{l}_{m}")
                    nc.scalar.activation(s_sb[:], zp[:, GL:4 * GL],
                                         mybir.ActivationFunctionType.Sigmoid)
                    i_ = s_sb[:, 0:GL]
                    f_ = s_sb[:, GL:2 * GL]
                    o_ = s_sb[:, 2 * GL:3 * GL]
                    ig = sbuf.tile([P, GL], f32, name=f"ig_{l}_{m}")
                    nc.vector.tensor_mul(ig[:], i_, g_sb[:])
                    fc = sbuf.tile([P, GL], f32, name=f"fc_{l}_{m}")
                    nc.vector.tensor_mul(fc[:], f_, cc[l][:, m * GL:(m + 1) * GL])
                    cn = sbuf.tile([P, GL], f32, name=f"cn_{l}_{m}")
                    nc.vector.tensor_add(cn[:], ig[:], fc[:])
                    nc.sync.dma_start(cn_d[l][:, m * GL:(m + 1) * GL], cn[:])
                    tc_ = sbuf.tile([P, GL], f32, name=f"tc_{l}_{m}")
                    nc.scalar.activation(tc_[:], cn[:],
                                         mybir.ActivationFunctionType.Tanh)
                    hn = sbuf.tile([P, GL], f32, name=f"hn_{l}_{m}")
                    nc.vector.tensor_mul(hn[:], o_, tc_[:])
                    nc.sync.dma_start(hn_d[l][:, m * GL:(m + 1) * GL], hn[:])
                    h16 = sbuf.tile([P, GL], f16, name=f"h16_{l}_{m}")
                    nc.vector.tensor_copy(h16[:], hn[:])
                    hn16.append(h16)

                # transpose h_new16 [2x (128,128)] -> hT_loc [128, 256]
                tp = psum_tp.tile([P, B], f16, name=f"tp_{l}", tag="tp")
                for m in range(MB):
                    nc.tensor.transpose(tp[:, m * P:(m + 1) * P],
                                        hn16[m][:], ident[:])
                hTl = sbuf.tile([P, B], f16, name=f"hTl_{l}")
                nc.vector.tensor_copy(hTl[:], tp[:])
                return hTl

            def gates_matmuls(l, lhs_x, lhs_x_k, lhs_h_tiles, start):
                """Emit the z matmuls for layer l into fresh psum tiles.
                lhs_x: SBUF tile holding K-major stationary input
                       ([P, kx*B] fp16), or None to skip (deferred W-part).
                start: True to start the psum accumulation group."""
                z_ps = []
                for m in range(MB):
                    zp = psum.tile([P, LW], f32, name=f"z_{l}_{m}", tag="z")
                    z_ps.append(zp)
                for m in range(MB):
                    zp = z_ps[m]
                    if start:
                        nc.tensor.matmul(zp[:], ones[:], bb[l][:],
                                         start=True, stop=False)
                    if lhs_x is not None:
                        for kk in range(lhs_x_k):
                            nc.tensor.matmul(
                                zp[:],
                                lhs_x[:, kk * B + m * P: kk * B + (m + 1) * P],
                                W[l][:, kk * LW:(kk + 1) * LW],
                                start=False, stop=False)
                    for kk in range(KH):
                        nc.tensor.matmul(
                            zp[:],
                            lhs_h_tiles[:, kk * B + m * P: kk * B + (m + 1) * P],
                            U[l][:, kk * LW:(kk + 1) * LW],
                            start=False, stop=(lhs_x is not None and kk == KH - 1))
                return z_ps

            def w_part(l, z_ps, lhs_tiles):
                for m in range(MB):
                    for kk in range(KH):
                        nc.tensor.matmul(
                            z_ps[m][:],
                            lhs_tiles[:, kk * B + m * P: kk * B + (m + 1) * P],
                            W[l][:, kk * LW:(kk + 1) * LW],
                            start=False, stop=(kk == KH - 1))

            # ---- cell 0: x-part + h0-part together
            z0 = gates_matmuls(0, xT, KI, hT[0], start=True)
            hT0 = cell(0, z0)
            nc.sync.dma_start(ag_in[0].ap()[:, :], hT0[:])
            nc.gpsimd.collective_compute(
                "AllGather", mybir.AluOpType.bypass, replica_groups=RG,
                ins=[ag_in[0].ap().opt()], outs=[ag_out[0].ap().opt()])

            # ---- cell 1: U-part first (input h1T known), W-part after AG0
            z1 = gates_matmuls(1, None, 0, hT[1], start=True)
            ag0_sb = sbuf.tile([P, KH * B], f16, name="ag0_sb")
            nc.sync.dma_start(
                ag0_sb[:].rearrange("p (a b) -> p a b", a=KH),
                ag_out[0].ap().rearrange("(a p) b -> p a b", p=P))
            w_part(1, z1, ag0_sb)
            hT1 = cell(1, z1)
            nc.sync.dma_start(ag_in[1].ap()[:, :], hT1[:])
            nc.gpsimd.collective_compute(
                "AllGather", mybir.AluOpType.bypass, replica_groups=RG,
                ins=[ag_in[1].ap().opt()], outs=[ag_out[1].ap().opt()])

            # ---- cell 2
            z2 = gates_matmuls(2, None, 0, hT[2], start=True)
            ag1_sb = sbuf.tile([P, KH * B], f16, name="ag1_sb")
            nc.sync.dma_start(
                ag1_sb[:].rearrange("p (a b) -> p a b", a=KH),
                ag_out[1].ap().rearrange("(a p) b -> p a b", p=P))
            w_part(2, z2, ag1_sb)
            hT2 = cell(2, z2)

            # ---- projection: partial logits = hT2_loc.T @ Wp_k
            for m in range(MB):
                lp = psum.tile([P, V], f32, name=f"lp_{m}", tag="z")
                nc.tensor.matmul(lp[:], hT2[:, m * P:(m + 1) * P], Wp[:],
                                 start=True, stop=True)
                lsb = sbuf.tile([P, V], f32, name=f"lsb_{m}")
                nc.vector.tensor_copy(lsb[:], lp[:])
                nc.sync.dma_start(logits_d[m * P:(m + 1) * P, :], lsb[:])

    nc.compile()
    return nc


def _get_compiled():
    global _compiled
    if _compiled is None:
        _compiled = _build()
    return _compiled


def _part_major(a, p=P):
    """[n*p, m] -> [p, n*m] partition-major interleave."""
    n = a.shape[0] // p
    return np.ascontiguousarray(
        a.reshape(n, p, a.shape[1]).transpose(1, 0, 2).reshape(p, n * a.shape[1]))


def _gate_cols(Wfull, k):
    """Columns of a [*, 4H] gate-matrix owned by core k, gate order g,i,f,o."""
    H_ = H
    idx = []
    for g in (2, 0, 1, 3):  # reference order i,f,g,o -> pick g first, then i,f,o
        idx.append(Wfull[:, g * H_ + k * GL:(g * H_ + (k + 1) * GL)])
    return np.concatenate(idx, axis=1)


def make_in_maps(x, h0, c0, h1, c1, h2, c2, W0, U0, b0, W1, U1, b1,
                 W2, U2, b2, Wp, bp):
    f16 = np.float16
    xT = _part_major(np.ascontiguousarray(x.T).astype(f16))
    hTs = [_part_major(np.ascontiguousarray(h.T).astype(f16))
           for h in (h0, h1, h2)]
    Ws = [W0, W1, W2]
    Us = [U0, U1, U2]
    bs = [b0, b1, b2]
    cs = [c0, c1, c2]

    in_maps = []
    for k in range(N_CORES):
        m = {"xT": xT}
        for l in range(3):
            m[f"h{l}T"] = hTs[l]
            m[f"W{l}"] = _part_major(_gate_cols(Ws[l], k).astype(f16))
            m[f"U{l}"] = _part_major(_gate_cols(Us[l], k).astype(f16))
            m[f"b{l}"] = _gate_cols(bs[l][None, :], k).astype(f16)
            m[f"c{l}"] = _part_major(
                np.ascontiguousarray(cs[l][:, k * GL:(k + 1) * GL]).astype(np.float32))
        m["Wp"] = np.ascontiguousarray(Wp[k * P:(k + 1) * P, :]).astype(f16)
        in_maps.append(m)
    return in_maps


def postprocess(per_core_outs, bp):
    """per_core_outs: list (len 8) of dicts with h{l}n/c{l}n/logits arrays."""
    def _unpart(a):
        # [p, MB*GL] -> [B(batch-major rows), GL]
        return a.reshape(P, MB, GL).transpose(1, 0, 2).reshape(B, GL)

    hn = [np.concatenate([_unpart(per_core_outs[k][f"h{l}n"])
                          for k in range(N_CORES)], axis=1) for l in range(3)]
    cn = [np.concatenate([_unpart(per_core_outs[k][f"c{l}n"])
                          for k in range(N_CORES)], axis=1) for l in range(3)]
    logits = np.sum([per_core_outs[k]["logits"] for k in range(N_CORES)], axis=0)
    logits = (logits + bp).astype(np.float32)
    return (logits, hn[0], cn[0], hn[1], cn[1], hn[2], cn[2])


def kernel(x, h0, c0, h1, c1, h2, c2, W0, U0, b0, W1, U1, b1, W2, U2, b2,
           Wp, bp):
    from concourse import bass_utils

    nc = _get_compiled()
    in_maps = make_in_maps(x, h0, c0, h1, c1, h2, c2, W0, U0, b0,
                           W1, U1, b1, W2, U2, b2, Wp, bp)
    res = bass_utils.run_bass_kernel_spmd(
        nc, in_maps, core_ids=list(range(N_CORES)),
        trace=bool(int(os.environ.get("LSTM_KERNEL_TRACE", "0"))))
    if res.exec_time_ns is not None:
        kernel.last_exec_time_ns = res.exec_time_ns
    return postprocess(res.results, bp)


kernel.last_exec_time_ns = None
